# revision 33
# baseline (speedup 1.0000x reference)
"""Trainium2 Bass kernel for a 2-layer LSTM (H=64) + FC head.

Problem: x [4096, 168, 19] f32 -> out [4096] f32
  h1 = LSTM0(x); h2 = LSTM1(h1); out = h2[:, -1, :] @ Wfc.T + bfc

Data-parallel over 8 NeuronCores (512 batch rows each). On each core
the batch is split into CH=3 independent chains (172/170/170 rows)
whose serial recurrences interleave on the engines. Layer 0 at time w
and layer 1 at time w-1 are computed together in one "wave" so every
op uses all 128 partitions (p0:64 = layer0, p64:128 = layer1).

All four gates use tanh only (sigmoid(x) = (1+tanh(x/2))/2 absorbed
into host-side weight prescaling), so the gate nonlinearity is ONE
activation op over the whole PSUM z tile per chain:

  tau = tanh(z banks G,F,I,O) in one ACT op (f32 PSUM -> bf16)
  m1 = tau_f * D              [Pool]
  u2 = (tau_i + 1) * tau_g    [DVE stt]  (= 2 i.g)
  v2 = D + m1                 [Pool]     (= 4 f.c)
  D' = (v2 * 0.5) + u2        [DVE stt]  (= 2 c')
  stc = tanh(0.5 * D')        [ACT]      (= tanh(c'))
  hm' = (tau_o + 1) * stc     [DVE stt]  (= 2 h)

with state D = 2c and hm = 2h.  Host prescale: gate columns f,i,o
x0.5 (tanh half-arg); all h-input rows x0.5 (hm carries 2h); Wfc x0.5.

z tiles are single-buffered (3 x 2 PSUM banks): the x-part mm1s of
wave w+1 only have a cheap WAR on wave w's gate-tanh read.  Only one
start=True per 2KB zero region (the neighbor slot zero-writes via its
pending bytes), so all four mm1s run before the h-dependent mm2s on
the in-order PE.
"""

import numpy as np

HIDDEN = 64
INPUT = 19
B = 4096
T = 168
NCORES = 8
BL = B // NCORES       # 512 per core
CBS = [172, 170, 170]  # batch rows per chain
CH = len(CBS)
COF = [0, 172, 342]    # chain offsets into the 512
CBMAX = 256            # padded PSUM slot width (bank-aligned)

# torch gate order rows: i(0:64) f(64:128) g(128:192) o(192:256)
# our bank (column-block) order: G, F, I, O
GATE_PERM = np.concatenate([
    np.arange(128, 192),  # g
    np.arange(64, 128),   # f
    np.arange(0, 64),     # i
    np.arange(192, 256),  # o
])
# per-bank gate scale: g unscaled, f/i/o halved (tanh half-argument)
GATE_SCALE = np.array([1.0, 0.5, 0.5, 0.5], np.float32)


def build_nc(steps=T):
    import concourse.bacc as bacc
    import concourse.tile as tile
    from concourse import mybir

    F32 = mybir.dt.float32
    BF16 = mybir.dt.bfloat16
    FMM = BF16
    FEL = BF16
    AF = mybir.ActivationFunctionType
    ALU = mybir.AluOpType

    nc = bacc.Bacc("TRN2", target_bir_lowering=False, debug=False,
                   num_devices=NCORES)

    xT = nc.dram_tensor("xT", [T, INPUT + 1, BL], FMM, kind="ExternalInput")
    w0x_d = nc.dram_tensor("w0x", [INPUT + 1, 512], FMM, kind="ExternalInput")
    whbig_d = nc.dram_tensor("whbig", [128, 512], FMM, kind="ExternalInput")
    wfc_d = nc.dram_tensor("wfc", [128, 1], FMM, kind="ExternalInput")
    zeros_d = nc.dram_tensor("zeros", [128, CBMAX], FMM, kind="ExternalInput")
    out = nc.dram_tensor("out", [1, BL], F32, kind="ExternalOutput")

    with tile.TileContext(nc) as tc:
        with (
            tc.tile_pool(name="const", bufs=1) as const,
            tc.tile_pool(name="state", bufs=1) as state,
            tc.tile_pool(name="work", bufs=6) as work,
            tc.tile_pool(name="xin", bufs=6) as xin,
            tc.tile_pool(name="zpool", bufs=CH, space="PSUM") as zpool,
            tc.tile_pool(name="fcpool", bufs=1, space="PSUM") as fcpool,
            tc.tile_pool(name="tpool", bufs=1, space="PSUM") as tpool,
        ):
            w0x = const.tile([INPUT + 1, 4, 128], FMM, tag="w0x", name="w0x")
            whbig = const.tile([128, 4, 128], FMM, tag="wh", name="whbig")
            wfc = const.tile([128, 1], FMM, tag="wfc", name="wfc")
            nc.sync.dma_start(w0x, w0x_d[:])
            nc.sync.dma_start(whbig, whbig_d[:])
            nc.sync.dma_start(wfc, wfc_d[:])

            # per-chain state: D = 2c, hm = [2h0; 2h1]
            C = [[state.tile([128, CBS[c]], FEL, tag=f"C{c}{p}",
                             name=f"C{c}{p}")
                  for p in (0, 1)] for c in range(CH)]
            hm = [[state.tile([128, CBS[c]], FMM, tag=f"hm{c}{p}",
                              name=f"hm{c}{p}")
                   for p in (0, 1)] for c in range(CH)]
            ones = const.tile([128, CBMAX], FEL, tag="ones", name="ones")
            nc.vector.memset(ones, 1.0)
            for c in range(CH):
                nc.vector.memset(C[c][0], 0.0)

            nwaves = steps + 1
            # per-chain stashes for the software-pipelined back half
            tols = [None] * CH

            def emit_front(c, w):
                """dma + matmuls + gate tanh + element-wise for (c, w)."""
                cur, nxt = w % 2, (w + 1) % 2
                cb = CBS[c]
                cs = slice(COF[c], COF[c] + cb)
                xt = xin.tile([INPUT + 1, cb], FMM, tag=f"x{c}", name=f"x{c}")
                nc.sync.dma_start(xt, xT[w % T, :, cs])

                # padded slots: each gate bank starts on a 1KB boundary
                # so no matmul crosses a PSUM bank.
                z = zpool.tile([128, 4, CBMAX], F32, tag="z", name=f"z{c}")
                for b in range(4):
                    nc.tensor.matmul(z[:, b, 0:cb], w0x[:, b, :],
                                     xt[:], start=(b % 2 == 0),
                                     stop=(w == 0 and b % 2 == 1),
                                     skip_group_check=True)
                if w > 0:
                    # wave 0 skips the h-part: hm is all zeros there
                    for b in range(4):
                        nc.tensor.matmul(z[:, b, 0:cb], whbig[:, b, :],
                                         hm[c][cur][:], start=False,
                                         stop=True, skip_group_check=True)

                # ONE tanh over all four gate banks (strided free AP).
                # Chain 2's tau lives in the spare PSUM bank: its gates
                # op is then PSUM->PSUM (143ns bubble instead of 185ns).
                if c == 2:
                    tau = tpool.tile([128, 4, cb], FEL, tag="taup",
                                     name=f"taup{c}")
                else:
                    tau = work.tile([128, 4, cb], FEL, tag=f"tau{c}",
                                    name=f"tau{c}")
                nc.scalar.activation(tau, z[:, :, 0:cb], AF.Tanh)

                tg, tf = tau[:, 0, :], tau[:, 1, :]
                ti, to = tau[:, 2, :], tau[:, 3, :]
                # m1 = tau_f * D             [Pool]
                m1 = work.tile([128, cb], FEL, tag=f"m1{c}", name=f"m1{c}")
                nc.gpsimd.tensor_mul(m1, tf, C[c][cur])
                # u2 = (1+tau_i).tau_g       [DVE]  (= 2 i.g)
                u2 = work.tile([128, cb], FEL, tag=f"u{c}", name=f"u{c}")
                nc.vector.scalar_tensor_tensor(
                    u2, ti, 1.0, tg, ALU.add, ALU.mult)
                # v2 = D + m1 = (1+tau_f).D  [Pool]  (= 4 f.c)
                v2 = work.tile([128, cb], FEL, tag=f"v{c}", name=f"v{c}")
                nc.gpsimd.tensor_add(v2, C[c][cur], m1)
                # D' = (v2 * 0.5) + u2       [DVE]  (= 2 c')
                nc.vector.scalar_tensor_tensor(
                    C[c][nxt], v2, 0.5, u2, ALU.mult, ALU.add)
                tols[c] = to

            def emit_back(c, w):
                """stc + h2 for (c, w); w==0 garbage reset for layer 1."""
                nxt = (w + 1) % 2
                # stc = tanh(0.5 * D') = tanh(c')   [ACT]
                stc = work.tile([128, CBS[c]], FEL, tag=f"stc{c}",
                                name=f"stc{c}")
                nc.scalar.activation(stc, C[c][nxt], AF.Tanh, scale=0.5)
                # hm' = (tau_o + 1) * stc = 2h   [DVE]
                nc.vector.scalar_tensor_tensor(
                    hm[c][nxt], tols[c], 1.0, stc, ALU.add, ALU.mult)
                if w == 0:
                    # wave 0's layer-1 half ran on garbage; reset it
                    nc.vector.memset(C[c][nxt][64:128], 0.0)
                    nc.sync.dma_start(hm[c][nxt][64:128],
                                      zeros_d[64:128, 0:CBS[c]])

            # Rotated emission: chain 1's back half is software-pipelined
            # one wave later, so the steady-state ACT order becomes
            # [g0, s1(w-1), g2, s0, g1, s2] -- each stc slots into the
            # gap between other chains' gate ops instead of the last
            # chain's stc blocking the next round of gates.
            for w in range(nwaves):
                emit_front(0, w)
                if w > 0:
                    emit_back(1, w - 1)
                emit_front(2, w)
                emit_back(0, w)
                emit_front(1, w)
                emit_back(2, w)
            emit_back(1, nwaves - 1)

            # --- FC head: out = Wfc . hm@last (bfc added on host) ---
            o_sb = work.tile([1, BL], F32, tag="osb", name="o_sb")
            for c in range(CH):
                pfc = fcpool.tile([1, CBS[c]], F32, tag="pfc",
                                  name=f"pfc{c}")
                nc.tensor.matmul(pfc, wfc, hm[c][nwaves % 2][:],
                                 start=True, stop=True)
                nc.scalar.activation(o_sb[:, COF[c]:COF[c] + CBS[c]], pfc,
                                     AF.Copy)
            nc.sync.dma_start(out[:], o_sb)

    nc.compile()
    return nc


def _mm_cast(a):
    import ml_dtypes
    return a.astype(ml_dtypes.bfloat16)


def make_in_maps(x, Wih0, Whh0, bih0, bhh0, Wih1, Whh1, bih1, bhh1, Wfc, bfc):
    """Shard + pre-transpose/concat/prescale inputs for the 8 cores."""
    p = GATE_PERM
    b0 = (bih0 + bhh0)[p].astype(np.float32)
    b1 = (bih1 + bhh1)[p].astype(np.float32)
    # w0x [20, 4, 128]: rows = [x features (19); ones]. Left cols =
    # [Wih0; b0] per gate, right cols = b1 on the ones row.
    # whbig [128, 4, 128]: left cols = [Whh0; 0], right cols =
    # [Wih1; Whh1] -- one K=128 matmul vs hm covers both layers.
    # Prescale: gate-bank scale sg (g:1, f/i/o:0.5) on everything;
    # whbig additionally x0.5 because hm carries 2h.
    w0x = np.zeros((INPUT + 1, 4, 128), np.float32)
    whbig = np.zeros((128, 4, 128), np.float32)
    for b in range(4):
        sg = GATE_SCALE[b]
        w0x[0:INPUT, b, 0:64] = sg * Wih0[p].T[:, b * 64:(b + 1) * 64]
        w0x[INPUT, b, 0:64] = sg * b0[b * 64:(b + 1) * 64]
        w0x[INPUT, b, 64:128] = sg * b1[b * 64:(b + 1) * 64]
        whbig[0:64, b, 0:64] = 0.5 * sg * Whh0[p].T[:, b * 64:(b + 1) * 64]
        whbig[0:64, b, 64:128] = 0.5 * sg * Wih1[p].T[:, b * 64:(b + 1) * 64]
        whbig[64:128, b, 64:128] = 0.5 * sg * Whh1[p].T[:, b * 64:(b + 1) * 64]
    wfcbig = np.zeros((128, 1), np.float32)
    wfcbig[64:128, 0] = 0.5 * Wfc.reshape(HIDDEN)
    base = {
        "w0x": _mm_cast(w0x.reshape(INPUT + 1, 512)),
        "whbig": _mm_cast(whbig.reshape(128, 512)),
        "wfc": _mm_cast(wfcbig),
        "zeros": _mm_cast(np.zeros((128, CBMAX), np.float32)),
    }
    xs = x.reshape(NCORES, BL, T, INPUT)
    in_maps = []
    for c in range(NCORES):
        m = dict(base)
        xt = np.empty((T, INPUT + 1, BL), np.float32)
        xt[:, 0:INPUT, :] = xs[c].transpose(1, 2, 0)
        xt[:, INPUT, :] = 1.0
        m["xT"] = _mm_cast(xt)
        in_maps.append(m)
    return in_maps


_CACHED_NC = None


def kernel(**inputs):
    global _CACHED_NC
    from concourse.bass_utils import run_bass_kernel_spmd

    if _CACHED_NC is None:
        _CACHED_NC = build_nc()
    nc = _CACHED_NC
    in_maps = make_in_maps(**inputs)
    res = run_bass_kernel_spmd(nc, in_maps, list(range(NCORES)))
    outs = [res.results[c]["out"].reshape(BL) for c in range(NCORES)]
    return np.concatenate(outs) + np.float32(inputs["bfc"][0])
